# revision 22
# baseline (speedup 1.0000x reference)
"""Causal self-attention (B=4, T=2048, D=1024, H=16) on 8 Trainium2 NeuronCores.

Sharding: batch x head-half. Core c handles batch b = c//2 and heads
hh..hh+7 where hh = 8*(c%2)  (tensor-parallel split of w_qkv output dim and
w_o input dim). Each core produces a partial o_proj output [2048, 1024] in
bf16; the host sums the two partials per batch in f32 (the 2-way
all-reduce).

Per-core kernel (all matmuls bf16, fp32 PSUM accumulate). Head pairs are
fused into [128, 1024] two-bank PSUM tiles throughout: phase 1 projects
Q^T,K^T head-pair-stacked [128, t] plus V in natural [t, dk] layout with an
appended ones column (rowsum trick, M=65 PV matmuls); phase 2 does causal
attention per (512-query tile, head pair) with S^T chunks, one strided exp
per chunk, static triangle masks on diagonal chunks; phase 3 o_proj.

Schedule (PE-saturating, sim ~233us vs ~283us for the phase-ordered
version):
  - input DMA interleaved by d-chunk across the SP and ACT queues; the
    first QKV phase runs dc-outer over 4 concurrent ec-group PSUMs so
    compute starts as soon as input chunk 0 lands.
  - the attention inner loop is software-pipelined (score kc+1 issued
    before PV kc) and a deadline-ordered filler queue weaves the
    second-half QKV / V / o_proj matmuls into the exp-bound attention
    stream so PE never idles; deadlines (qt, pr, kc) guarantee a filler
    group's output is emitted before the first attention chunk that
    reads it.
  - engine budget: exp on ACT; PSUM->SBUF copies on DVE (ACT during
    projection phases); triangle masks on DVE (HW A/B: Pool's strict
    per-Q7 FIFO made the exp->mask->PV chain ~100us slower); normalize
    muls and partition broadcast on Pool (no PSUM port, so everything it
    touches is SBUF); PSUM accumulators are freed early via a single
    [65,1024] evacuation copy per head pair; output DMA on SP, bf16.
  - exp instructions are flat contiguous [128,1024] for full-width
    chunks (lo==0); the strided 3-d AP form is only used for partial
    diagonal chunks. HW A/B: the strided exp cost ~35%/inst more on ACT
    and ~100us end-to-end.
  - PSUM budget (8 banks): 2x score tiles [128,1024] + 1 PV-pair tile
    [128,1024] + 2 half-bank filler tiles [128,512] with rotating tags.
"""
import numpy as np
import ml_dtypes

B, T, D, H = 4, 2048, 1024, 16
DK = D // H          # 64
HPC = 8              # heads per core
NCORES = 8
NQT = T // 512       # 4
NKC = T // 128       # 16

_cache = {}
FLAT_EXP = False


def _emit_inputs(nc, big, xt_d, wq_d, wo_d, skip_dma=False):
    """Input tiles + their DMA loads. d-chunk k (x half + qk-weight half)
    arrives ~in lockstep on two queues so the dc-outer first QKV phase is
    never input-starved. chunk 0 is further split in half so the very first
    matmul can start a few hundred ns earlier."""
    import concourse.mybir as mybir
    bf16 = mybir.dt.bfloat16

    xta = [big.tile([128, 1024], bf16, tag=f"xta{dc}", name=f"xta{dc}")
           for dc in range(8)]
    xtb = [big.tile([128, 1024], bf16, tag=f"xtb{dc}", name=f"xtb{dc}")
           for dc in range(8)]
    wqk = [big.tile([128, 1024], bf16, tag=f"wqk{dc}", name=f"wqk{dc}")
           for dc in range(8)]
    wv = [big.tile([128, 512], bf16, tag=f"wv{dc}", name=f"wv{dc}")
          for dc in range(8)]
    wos = [big.tile([128, D], bf16, tag=f"wo{pr}", name=f"wos{pr}")
           for pr in range(4)]
    if skip_dma:
        for dc in range(8):
            nc.gpsimd.memset(xta[dc][:], 0.01)
            nc.gpsimd.memset(xtb[dc][:], 0.01)
            nc.gpsimd.memset(wqk[dc][:], 0.01)
            nc.gpsimd.memset(wv[dc][:], 0.01)
        for pr in range(4):
            nc.gpsimd.memset(wos[pr][:], 0.01)
        return xta, xtb, wqk, wv, wos
    nc.gpsimd.dma_start(wqk[0][:, 0:512], wq_d[0, :, 0:512])
    for dc in range(8):
        if dc == 0:
            nc.sync.dma_start(xta[0][:, 0:512], xt_d[0, :, 0:512])
            nc.sync.dma_start(xta[0][:, 512:1024], xt_d[0, :, 512:1024])
            nc.scalar.dma_start(wqk[0][:, 512:1024], wq_d[0, :, 512:1024])
            continue
        nc.sync.dma_start(xta[dc][:], xt_d[dc, :, 0:1024])
        nc.scalar.dma_start(wqk[dc][:], wq_d[dc, :, 0:1024])
    for dc in range(8):
        nc.sync.dma_start(xtb[dc][:], xt_d[dc, :, 1024:2048])
        nc.scalar.dma_start(wv[dc][:], wq_d[dc, :, 1024:1536])
    for pr in range(4):
        nc.scalar.dma_start(wos[pr][:], wo_d[pr])
    return xta, xtb, wqk, wv, wos


def _emit(nc, tc, pools, dram, opts=()):
    import concourse.mybir as mybir

    bf16 = mybir.dt.bfloat16
    f32 = mybir.dt.float32
    Exp = mybir.ActivationFunctionType.Exp
    Copy = mybir.ActivationFunctionType.Copy
    cst, big, work, norm, stp, psa, psb, psf = pools
    xt_d, wq_d, wo_d, out_d, masks = dram
    flat_exp = FLAT_EXP or ('flat_exp' in opts)
    nomask = 'nomask' in opts
    nonorm = 'nonorm' in opts
    nopv = 'nopv' in opts
    # defaults chosen by HW A/B (same-batch): flat exp for full-width chunks
    # and the diagonal mask on DVE instead of Pool are each ~100us faster.
    oneflat = 'no_oneflat' not in opts
    dvemask = 'no_dvemask' not in opts
    mask2 = 'mask2' in opts
    dvemask2 = 'dvemask2' in opts
    dvenorm = 'dvenorm' in opts
    if 'rlow' in opts:
        rates = (1.2, 1.2, 1.0, 1.0)
    elif 'rhigh' in opts:
        rates = (1.8, 1.8, 1.5, 1.5)
    else:
        rates = (1.5, 1.5, 1.2, 1.2)

    qkv_only = 'qkv_only' in opts
    attn_only = 'attn_only' in opts

    pre = getattr(tc, '_kx_preloaded', {})
    if attn_only:
        if 'qkvt' in pre:
            qk, vt = pre['qkvt']
        else:
            qk = big.tile([128, 8, T], bf16, tag="qk")
            vt = big.tile([128, NKC, HPC, DK + 1], bf16, tag="vt")
            nc.gpsimd.memset(qk[:], 0.02)
            nc.gpsimd.memset(vt[:], 1.0)
        ob = big.tile([128, 4, T], bf16, tag="ob")
        xta = xtb = wqk = wv = wos = None
    else:
        if 'inputs' in pre:
            xta, xtb, wqk, wv, wos = pre['inputs']
        else:
            xta, xtb, wqk, wv, wos = _emit_inputs(nc, big, xt_d, wq_d, wo_d)
        qk = big.tile([128, 8, T], bf16, tag="qk")
        vt = big.tile([128, NKC, HPC, DK + 1], bf16, tag="vt")
        ob = big.tile([128, 4, T], bf16, tag="ob")
        nc.gpsimd.memset(vt[:, :, :, DK], 1.0)


    # ---- filler machinery ----------------------------------------------
    # Filler groups are sequences of ~1-matmul closures writing one
    # [128,512] PSUM bank allocated from two rotating tags (filA/filB) so
    # consecutive groups double-buffer. Groups carry a deadline
    # (qt, pr, kc): the attention chunk whose emission needs their output;
    # force() drains all due groups, pump() feeds them in at a given rate
    # to cover PE idle while ACT runs the exps. The queue is kept sorted
    # by deadline (None = last) and only whole groups are ever reordered.
    groups = []        # list of [deadline, [closures...]]
    reserve = []
    pump_acc = [0.0]
    tag_rot = [0]

    def fil_tile(name):
        tag = ("filA", "filB")[tag_rot[0] & 1]
        tag_rot[0] += 1
        return psf.tile([128, 512], f32, tag=tag, name=name)

    def enqueue(deadline, closures):
        i = len(groups)
        if deadline is not None:
            while i > 0 and (groups[i - 1][0] is None
                             or groups[i - 1][0] > deadline):
                i -= 1
        groups.insert(i, [deadline, list(closures)])

    def _pop_one():
        g = groups[0]
        g[1].pop(0)()
        if not g[1]:
            groups.pop(0)

    def pump(n=1.0):
        pump_acc[0] += n
        while pump_acc[0] >= 1.0 and groups:
            _pop_one()
            pump_acc[0] -= 1.0

    def force(key):
        while groups and groups[0][0] is not None and groups[0][0] <= key:
            _pop_one()

    def drain():
        while groups:
            _pop_one()

    # ---- projection emitters -------------------------------------------
    # first-half QKV (t 0:1024): dc-outer over 4 live ec-groups so compute
    # starts as soon as input chunk 0 lands. ec-groups 0..2 use the three
    # [128,1024] PSUM tiles; ec-group 3 uses the two half-bank filler tiles.
    def emit_qkv_first(ec_group):
        t01 = psa.tile([128, 1024], f32, tag="s", name=f"qA{ec_group[0]}")
        t1 = psa.tile([128, 1024], f32, tag="s", name=f"qA{ec_group[1]}")
        t2 = psb.tile([128, 1024], f32, tag="pv", name=f"qA{ec_group[2]}")
        t3a = fil_tile(f"qA{ec_group[3]}a")
        t3b = fil_tile(f"qA{ec_group[3]}b")
        views = {ec_group[0]: (t01[:, 0:512], t01[:, 512:1024]),
                 ec_group[1]: (t1[:, 0:512], t1[:, 512:1024]),
                 ec_group[2]: (t2[:, 0:512], t2[:, 512:1024]),
                 ec_group[3]: (t3a[:], t3b[:])}
        for dc in range(8):
            for ec in ec_group:
                for half in range(2):
                    nc.tensor.matmul(
                        views[ec][half],
                        wqk[dc][:, ec * 128:(ec + 1) * 128],
                        xta[dc][:, half * 512:(half + 1) * 512],
                        start=(dc == 0), stop=(dc == 7),
                    )
        for gi, ec in enumerate(ec_group):
            if gi < 3:
                src = (t01, t1, t2)[gi]
                nc.scalar.activation(qk[:, ec, 0:1024], src[:], Copy)
            else:
                nc.scalar.activation(qk[:, ec, 0:512], t3a[:], Copy)
                nc.scalar.activation(qk[:, ec, 512:1024], t3b[:], Copy)

    # V projection for t rows [256*tq, 256*tq+256), first half (phase)
    def emit_v(tq):
        ps = psa.tile([128, 1024], f32, tag="s", name=f"vps{tq}")
        for half in range(2):
            tt = 2 * tq + half
            for dc in range(8):
                nc.tensor.matmul(
                    ps[:, half * 512:(half + 1) * 512],
                    xta[dc][:, (tt % 8) * 128:(tt % 8) * 128 + 128],
                    wv[dc][:],
                    start=(dc == 0), stop=(dc == 7),
                )
        nc.scalar.activation(
            vt[:, 2 * tq:2 * tq + 2, :, 0:DK],
            ps[:].rearrange("p (t h d) -> p t h d", t=2, d=DK), Copy)

    # ---- filler group builders (second halves + o_proj) -----------------
    def enqueue_qkv_second(ec, deadline=None):
        for half in range(2):
            box = {}

            def mk(dc, half=half, box=box):
                def f():
                    if dc == 0:
                        box["ps"] = fil_tile(f"qB{ec}h{half}")
                    nc.tensor.matmul(
                        box["ps"][:],
                        wqk[dc][:, ec * 128:(ec + 1) * 128],
                        xtb[dc][:, half * 512:(half + 1) * 512],
                        start=(dc == 0), stop=(dc == 7),
                    )
                    if dc == 7:
                        nc.vector.tensor_copy(
                            qk[:, ec, 1024 + half * 512:1536 + half * 512],
                            box["ps"][:])
                return f
            enqueue(deadline, [mk(dc) for dc in range(8)])

    def enqueue_v_second(tq, deadline=None):
        for half in range(2):
            tt = 2 * tq + half
            box = {}

            def mk(dc, tt=tt, box=box):
                def f():
                    if dc == 0:
                        box["ps"] = fil_tile(f"vB{tt}")
                    nc.tensor.matmul(
                        box["ps"][:],
                        xtb[dc][:, (tt % 8) * 128:(tt % 8) * 128 + 128],
                        wv[dc][:],
                        start=(dc == 0), stop=(dc == 7),
                    )
                    if dc == 7:
                        nc.vector.tensor_copy(
                            vt[:, tt, :, 0:DK],
                            box["ps"][:].rearrange("p (h d) -> p h d", d=DK))
                return f
            enqueue(deadline, [mk(dc) for dc in range(8)])

    def oproj_half_closures(tt, eh, act_copy=False):
        box = {}

        def mk(pr):
            def f():
                if pr == 0:
                    box["po"] = fil_tile(f"po{tt}e{eh}")
                nc.tensor.matmul(
                    box["po"][:],
                    ob[:, pr, tt * 128:(tt + 1) * 128],
                    wos[pr][:, eh * 512:(eh + 1) * 512],
                    start=(pr == 0), stop=(pr == 3))
                if pr == 3:
                    ot = work.tile([128, 512], bf16, tag="ot")
                    if act_copy:
                        nc.scalar.activation(ot[:], box["po"][:], Copy)
                    else:
                        nc.vector.tensor_copy(ot[:], box["po"][:])
                    nc.sync.dma_start(
                        out_d[tt * 128:(tt + 1) * 128,
                              eh * 512:(eh + 1) * 512], ot[:])
            return f
        return [mk(pr) for pr in range(4)]

    def enqueue_oproj(qt, hold=0):
        for tt in range(4 * qt, 4 * qt + 4):
            if hold and tt >= 4 * qt + 4 - hold:
                reserve.append(tt)
                continue
            for eh in range(2):
                enqueue(None, oproj_half_closures(tt, eh))

    # ---- attention for one 512-query tile, software-pipelined -----------
    def emit_attention(qt, rate=1.0):
        nkc = 4 * qt + 4
        for pr in range(4):          # head pair (2*pr, 2*pr+1)
            pvp = psb.tile([128, 1024], f32, tag="pv")
            sts = {}

            def emit_score(kc):
                i = kc - 4 * qt
                lo = max(i, 0) * 128
                sp = psa.tile([128, 1024], f32, tag="s")
                nc.tensor.matmul(
                    sp[:, lo:512], qk[0:64, 4 + pr, kc * 128:(kc + 1) * 128],
                    qk[0:64, pr, qt * 512 + lo:(qt + 1) * 512],
                    start=True, stop=True, tile_position=(0, 0))
                nc.tensor.matmul(
                    sp[:, 512 + lo:1024],
                    qk[64:128, 4 + pr, kc * 128:(kc + 1) * 128],
                    qk[64:128, pr, qt * 512 + lo:(qt + 1) * 512],
                    start=True, stop=True, tile_position=(64, 0))
                st = stp.tile([128, 1024], bf16, tag="st")
                if flat_exp:
                    nc.scalar.activation(st[:, lo:512], sp[:, lo:512],
                                         Exp, scale=0.125)
                    nc.scalar.activation(st[:, 512 + lo:1024],
                                         sp[:, 512 + lo:1024],
                                         Exp, scale=0.125)
                elif oneflat and lo == 0:
                    # full-width chunk: both head halves span their whole 512
                    # columns, so one flat contiguous [128,1024] exp (the
                    # strided 3-d AP costs ~35% more on ACT and was the
                    # dominant drag on the attention stream).
                    nc.scalar.activation(st[:], sp[:], Exp, scale=0.125)
                else:
                    sp3 = sp[:].rearrange("p (h q) -> p h q", h=2)[:, :, lo:]
                    st3 = st[:].rearrange("p (h q) -> p h q", h=2)[:, :, lo:]
                    nc.scalar.activation(st3, sp3, Exp, scale=0.125)
                if i >= 0 and not nomask:
                    # diagonal chunk: mask the triangular blocks
                    if mask2 or dvemask2:
                        # two contiguous [128,128] muls (no strided AP)
                        eng = nc.vector if dvemask2 else nc.gpsimd
                        for hh in range(2):
                            blk = st[:, hh * 512 + lo:hh * 512 + lo + 128]
                            eng.tensor_mul(blk, blk, masks[:, hh, :])
                    else:
                        std = st[:].rearrange(
                            "p (h q) -> p h q", h=2)[:, :, lo:lo + 128]
                        eng = nc.vector if dvemask else nc.gpsimd
                        eng.tensor_mul(std, std, masks[:])
                sts[kc] = st

            def emit_pv(kc):
                i = kc - 4 * qt
                lo = max(i, 0) * 128
                st = sts.pop(kc)
                if nopv:
                    return
                nc.tensor.matmul(
                    pvp[0:65, lo:512], vt[:, kc, 2 * pr, :], st[:, lo:512],
                    start=(kc == 0), stop=(kc == nkc - 1))
                nc.tensor.matmul(
                    pvp[0:65, 512 + lo:1024], vt[:, kc, 2 * pr + 1, :],
                    st[:, 512 + lo:1024],
                    start=(kc == 0), stop=(kc == nkc - 1))

            force((qt, pr, 0))
            emit_score(0)
            for kc in range(nkc):
                force((qt, pr, kc))
                if kc + 1 < nkc:
                    emit_score(kc + 1)
                pump(rate)
                emit_pv(kc)

            # normalize: evacuate pvp in one copy (frees the PSUM bank pair
            # fast), then 1/rowsum on the SBUF copy. Pool has no PSUM port,
            # so: DVE evac -> Pool sd stage -> DVE recip -> Pool broadcast
            # -> Pool muls.
            if nonorm or nopv:
                pump(4)
                continue
            pvc = work.tile([65, 1024], f32, tag="pvc")
            sd = norm.tile([1, 1024], f32, tag="sd")
            if qt == 3 and pr == 3:
                nc.scalar.activation(sd[:], pvp[64:65, :], Copy)
                nc.vector.tensor_copy(pvc[:], pvp[0:65, :])
            else:
                nc.vector.tensor_copy(pvc[:], pvp[0:65, :])
                if dvesd:
                    nc.vector.tensor_copy(sd[:], pvc[64:65, :])
                elif actsd:
                    nc.scalar.activation(sd[:], pvc[64:65, :], Copy)
                else:
                    nc.gpsimd.tensor_copy(sd[:], pvc[64:65, :])
            rc = norm.tile([1, 1024], f32, tag="rc")
            nc.vector.reciprocal_approx_fast(rc[:], sd[:])
            rb = norm.tile([64, 1024], f32, tag="rb")
            nc.gpsimd.partition_broadcast(rb[:], rc[:])
            for hh in range(2):
                meng = nc.vector if dvenorm else nc.gpsimd
                meng.tensor_mul(
                    ob[64 * hh:64 * hh + 64, pr, qt * 512:(qt + 1) * 512],
                    pvc[0:64, hh * 512:(hh + 1) * 512],
                    rb[:, hh * 512:(hh + 1) * 512])
            pump(4)

    # ---- schedule ----
    if attn_only:
        for qt in range(4):
            emit_attention(qt, rate=1.0)
        ot = work.tile([128, 512], bf16, tag="ot")
        nc.vector.tensor_copy(ot[:], ob[:, 0, 0:512])
        nc.sync.dma_start(out_d[0:128, 0:512], ot[:])
        return
    emit_qkv_first([0, 1, 2, 3])     # Q heads 0..7, t 0:1024
    emit_qkv_first([4, 5, 6, 7])     # K heads 0..7, t 0:1024
    for tq in range(4):              # V rows 0:1024
        emit_v(tq)
    if qkv_only:
        for ec in range(8):
            enqueue_qkv_second(ec)
        for tq in range(4, 8):
            enqueue_v_second(tq)
        drain()
        ot = work.tile([128, 512], bf16, tag="ot")
        nc.vector.tensor_copy(ot[:], qk[:, 0, 0:512])
        nc.sync.dma_start(out_d[0:128, 0:512], ot[:])
        return

    # Q cols 1024:1536 must exist when attention(2) starts; K pair p's
    # second half is first read at attention(2) pair p chunk 8; vt rows
    # 8:12 at attention(2) pair 0 chunks 8/10; vt rows 12:16 at
    # attention(3) pair 0 chunks 12/14. Everything else floats free.
    for ec in range(4):
        enqueue_qkv_second(ec, deadline=(2, 0, 0))
    enqueue_qkv_second(4, deadline=(2, 0, 7))
    enqueue_v_second(4, deadline=(2, 0, 8))
    enqueue_v_second(5, deadline=(2, 0, 10))
    enqueue_qkv_second(5, deadline=(2, 1, 7))
    enqueue_qkv_second(6, deadline=(2, 2, 7))
    enqueue_qkv_second(7, deadline=(2, 3, 7))
    enqueue_v_second(6, deadline=(3, 0, 12))
    enqueue_v_second(7, deadline=(3, 0, 14))

    emit_attention(0, rate=rates[0])
    enqueue_oproj(0)
    emit_attention(1, rate=rates[1])
    enqueue_oproj(1)
    emit_attention(2, rate=rates[2])
    enqueue_oproj(2, hold=3)
    emit_attention(3, rate=rates[3])
    drain()
    # tail o_proj (held tiles + qt=3): full-width PSUM tiles from the
    # now-free big pools -- no dependency on the half-bank filler slots,
    # whose recycling is gated by laggy DVE copies. Copies alternate
    # ACT/DVE so they pipeline; psb tiles come later so the last head
    # pair's PSUM evacuation has finished.
    tail_tts = list(reserve) + [12, 13, 14, 15]
    reserve.clear()
    tail_pools = [(psa, "s"), (psa, "s"), (psb, "pv")]
    for k, tt in enumerate(tail_tts):
        if k >= len(tail_tts) - 2:
            # last two tiles: eh-split into half-bank PSUMs so each
            # evacuation copy starts four matmuls earlier
            for eh in range(2):
                po = fil_tile(f"po3_{tt}e{eh}")
                for pr in range(4):
                    nc.tensor.matmul(
                        po[:],
                        ob[:, pr, tt * 128:(tt + 1) * 128],
                        wos[pr][:, eh * 512:(eh + 1) * 512],
                        start=(pr == 0), stop=(pr == 3))
                ot = work.tile([128, 512], bf16, tag="ot")
                if eh == 0:
                    nc.scalar.activation(ot[:], po[:], Copy)
                else:
                    nc.vector.tensor_copy(ot[:], po[:])
                q = nc.sync if eh == 0 else nc.gpsimd
                q.dma_start(
                    out_d[tt * 128:(tt + 1) * 128, eh * 512:(eh + 1) * 512],
                    ot[:])
            continue
        pool, tag = tail_pools[k % 3]
        po = pool.tile([128, 1024], f32, tag=tag, name=f"po3_{tt}")
        for eh in range(2):
            for pr in range(4):
                nc.tensor.matmul(
                    po[:, eh * 512:(eh + 1) * 512],
                    ob[:, pr, tt * 128:(tt + 1) * 128],
                    wos[pr][:, eh * 512:(eh + 1) * 512],
                    start=(pr == 0), stop=(pr == 3))
        for eh in range(2):
            ot = work.tile([128, 512], bf16, tag="ot")
            if k % 2 == 0:
                nc.scalar.activation(ot[:], po[:, eh * 512:(eh + 1) * 512],
                                     Copy)
            else:
                nc.vector.tensor_copy(ot[:], po[:, eh * 512:(eh + 1) * 512])
            q = nc.sync if (2 * k + eh) % 2 == 0 else nc.gpsimd
            q.dma_start(
                out_d[tt * 128:(tt + 1) * 128, eh * 512:(eh + 1) * 512],
                ot[:])


def _emit_hsplit(nc, tc, pools, dram, opts=()):
    """Head-split attention: per-head 1-bank score/PV PSUM tiles so the two
    heads of a pair form independent score->exp->PV streams; cross-engine
    semaphore latency hides under the sibling stream. PSUM: 4x sp [128,512]
    (psa) + 2x pvp [128,512] (psb pva/pvb) + 2x filler (psf) = 8 banks."""
    import concourse.mybir as mybir

    bf16 = mybir.dt.bfloat16
    f32 = mybir.dt.float32
    Exp = mybir.ActivationFunctionType.Exp
    Copy = mybir.ActivationFunctionType.Copy
    cst, big, work, norm, stp, psa, psb, psf = pools
    xt_d, wq_d, wo_d, out_d, masks = dram
    qkv_only = 'qkv_only' in opts
    attn_only = 'attn_only' in opts

    pre = getattr(tc, '_kx_preloaded', {})
    if attn_only:
        if 'qkvt' in pre:
            qk, vt = pre['qkvt']
        else:
            qk = big.tile([128, 8, T], bf16, tag="qk")
            vt = big.tile([128, NKC, HPC, DK + 1], bf16, tag="vt")
            nc.gpsimd.memset(qk[:], 0.02)
            nc.gpsimd.memset(vt[:], 1.0)
        ob = big.tile([128, 4, T], bf16, tag="ob")
        xta = xtb = wqk = wv = wos = None
    else:
        if 'inputs' in pre:
            xta, xtb, wqk, wv, wos = pre['inputs']
        else:
            xta, xtb, wqk, wv, wos = _emit_inputs(nc, big, xt_d, wq_d, wo_d)
        qk = big.tile([128, 8, T], bf16, tag="qk")
        vt = big.tile([128, NKC, HPC, DK + 1], bf16, tag="vt")
        ob = big.tile([128, 4, T], bf16, tag="ob")
        nc.gpsimd.memset(vt[:, :, :, DK], 1.0)

    # ---- filler machinery (identical to _emit) -------------------------
    groups = []
    reserve = []
    pump_acc = [0.0]
    tag_rot = [0]

    def fil_tile(name):
        tag = ("filA", "filB")[tag_rot[0] & 1]
        tag_rot[0] += 1
        return psf.tile([128, 512], f32, tag=tag, name=name)

    def enqueue(deadline, closures):
        i = len(groups)
        if deadline is not None:
            while i > 0 and (groups[i - 1][0] is None
                             or groups[i - 1][0] > deadline):
                i -= 1
        groups.insert(i, [deadline, list(closures)])

    def _pop_one():
        g = groups[0]
        g[1].pop(0)()
        if not g[1]:
            groups.pop(0)

    def pump(n=1.0):
        pump_acc[0] += n
        while pump_acc[0] >= 1.0 and groups:
            _pop_one()
            pump_acc[0] -= 1.0

    def force(key):
        while groups and groups[0][0] is not None and groups[0][0] <= key:
            _pop_one()

    def drain():
        while groups:
            _pop_one()

    def sp_tile(name):
        return psa.tile([128, 512], f32, tag="s", name=name)

    # ---- projection emitters (1-bank tiles) ----------------------------
    def emit_qkv_first(ec_group):
        slots = [sp_tile(f"qA{ec_group[0]}a"), sp_tile(f"qA{ec_group[0]}b"),
                 sp_tile(f"qA{ec_group[1]}a"), sp_tile(f"qA{ec_group[1]}b"),
                 psb.tile([128, 512], f32, tag="pva", name=f"qA{ec_group[2]}a"),
                 psb.tile([128, 512], f32, tag="pvb", name=f"qA{ec_group[2]}b"),
                 fil_tile(f"qA{ec_group[3]}a"), fil_tile(f"qA{ec_group[3]}b")]
        for dc in range(8):
            for gi, ec in enumerate(ec_group):
                for half in range(2):
                    nc.tensor.matmul(
                        slots[2 * gi + half][:],
                        wqk[dc][:, ec * 128:(ec + 1) * 128],
                        xta[dc][:, half * 512:(half + 1) * 512],
                        start=(dc == 0), stop=(dc == 7),
                    )
        for gi, ec in enumerate(ec_group):
            for half in range(2):
                dst = qk[:, ec, half * 512:(half + 1) * 512]
                src = slots[2 * gi + half][:]
                if gi % 2 == 0:
                    nc.scalar.activation(dst, src, Copy)
                else:
                    nc.vector.tensor_copy(dst, src)

    def emit_v(tq):
        for half in range(2):
            tt = 2 * tq + half
            ps = sp_tile(f"vps{tt}")
            for dc in range(8):
                nc.tensor.matmul(
                    ps[:],
                    xta[dc][:, (tt % 8) * 128:(tt % 8) * 128 + 128],
                    wv[dc][:],
                    start=(dc == 0), stop=(dc == 7),
                )
            nc.scalar.activation(
                vt[:, tt, :, 0:DK],
                ps[:].rearrange("p (h d) -> p h d", d=DK), Copy)

    # ---- filler group builders (same as _emit) -------------------------
    def enqueue_qkv_second(ec, deadline=None):
        for half in range(2):
            box = {}

            def mk(dc, half=half, box=box):
                def f():
                    if dc == 0:
                        box["ps"] = fil_tile(f"qB{ec}h{half}")
                    nc.tensor.matmul(
                        box["ps"][:],
                        wqk[dc][:, ec * 128:(ec + 1) * 128],
                        xtb[dc][:, half * 512:(half + 1) * 512],
                        start=(dc == 0), stop=(dc == 7),
                    )
                    if dc == 7:
                        nc.vector.tensor_copy(
                            qk[:, ec, 1024 + half * 512:1536 + half * 512],
                            box["ps"][:])
                return f
            enqueue(deadline, [mk(dc) for dc in range(8)])

    def enqueue_v_second(tq, deadline=None):
        for half in range(2):
            tt = 2 * tq + half
            box = {}

            def mk(dc, tt=tt, box=box):
                def f():
                    if dc == 0:
                        box["ps"] = fil_tile(f"vB{tt}")
                    nc.tensor.matmul(
                        box["ps"][:],
                        xtb[dc][:, (tt % 8) * 128:(tt % 8) * 128 + 128],
                        wv[dc][:],
                        start=(dc == 0), stop=(dc == 7),
                    )
                    if dc == 7:
                        nc.vector.tensor_copy(
                            vt[:, tt, :, 0:DK],
                            box["ps"][:].rearrange("p (h d) -> p h d", d=DK))
                return f
            enqueue(deadline, [mk(dc) for dc in range(8)])

    def oproj_half_closures(tt, eh, act_copy=False):
        box = {}

        def mk(pr):
            def f():
                if pr == 0:
                    box["po"] = fil_tile(f"po{tt}e{eh}")
                nc.tensor.matmul(
                    box["po"][:],
                    ob[:, pr, tt * 128:(tt + 1) * 128],
                    wos[pr][:, eh * 512:(eh + 1) * 512],
                    start=(pr == 0), stop=(pr == 3))
                if pr == 3:
                    ot = work.tile([128, 512], bf16, tag="ot")
                    if act_copy:
                        nc.scalar.activation(ot[:], box["po"][:], Copy)
                    else:
                        nc.vector.tensor_copy(ot[:], box["po"][:])
                    nc.sync.dma_start(
                        out_d[tt * 128:(tt + 1) * 128,
                              eh * 512:(eh + 1) * 512], ot[:])
            return f
        return [mk(pr) for pr in range(4)]

    def enqueue_oproj(qt, hold=0):
        for tt in range(4 * qt, 4 * qt + 4):
            if hold and tt >= 4 * qt + 4 - hold:
                reserve.append(tt)
                continue
            for eh in range(2):
                enqueue(None, oproj_half_closures(tt, eh))

    mask2d = masks[:, 0, :]

    # ---- attention: head-split streams ---------------------------------
    def emit_attention(qt, rate=1.0):
        nkc = 4 * qt + 4
        for pr in range(4):
            pva = psb.tile([128, 512], f32, tag="pva", name=f"pva{qt}_{pr}")
            pvb = psb.tile([128, 512], f32, tag="pvb", name=f"pvb{qt}_{pr}")
            sts = {}

            def emit_score(kc, pva=pva, pvb=pvb, sts=sts):
                i = kc - 4 * qt
                lo = max(i, 0) * 128
                spa = sp_tile(f"sa{qt}{pr}_{kc}")
                spb = sp_tile(f"sb{qt}{pr}_{kc}")
                nc.tensor.matmul(
                    spa[:, lo:512], qk[0:64, 4 + pr, kc * 128:(kc + 1) * 128],
                    qk[0:64, pr, qt * 512 + lo:(qt + 1) * 512],
                    start=True, stop=True, tile_position=(0, 0))
                nc.tensor.matmul(
                    spb[:, lo:512],
                    qk[64:128, 4 + pr, kc * 128:(kc + 1) * 128],
                    qk[64:128, pr, qt * 512 + lo:(qt + 1) * 512],
                    start=True, stop=True, tile_position=(64, 0))
                sta = stp.tile([128, 512], bf16, tag="st", name=f"ta{kc}")
                stb = stp.tile([128, 512], bf16, tag="st", name=f"tb{kc}")
                nc.scalar.activation(sta[:, lo:512], spa[:, lo:512], Exp,
                                     scale=0.125)
                nc.scalar.activation(stb[:, lo:512], spb[:, lo:512], Exp,
                                     scale=0.125)
                if i >= 0:
                    nc.gpsimd.tensor_mul(sta[:, lo:lo + 128],
                                         sta[:, lo:lo + 128], mask2d)
                    nc.gpsimd.tensor_mul(stb[:, lo:lo + 128],
                                         stb[:, lo:lo + 128], mask2d)
                sts[kc] = (sta, stb)

            def emit_pv(kc, pva=pva, pvb=pvb, sts=sts):
                i = kc - 4 * qt
                lo = max(i, 0) * 128
                sta, stb = sts.pop(kc)
                nc.tensor.matmul(
                    pva[0:65, lo:512], vt[:, kc, 2 * pr, :], sta[:, lo:512],
                    start=(kc == 0), stop=(kc == nkc - 1))
                nc.tensor.matmul(
                    pvb[0:65, lo:512], vt[:, kc, 2 * pr + 1, :],
                    stb[:, lo:512],
                    start=(kc == 0), stop=(kc == nkc - 1))

            force((qt, pr, 0))
            emit_score(0)
            for kc in range(nkc):
                force((qt, pr, kc))
                if kc + 1 < nkc:
                    emit_score(kc + 1)
                pump(rate)
                emit_pv(kc)

            pvc = work.tile([65, 1024], f32, tag="pvc")
            sd = norm.tile([1, 1024], f32, tag="sd")
            nc.vector.tensor_copy(pvc[:, 0:512], pva[0:65, :])
            nc.vector.tensor_copy(pvc[:, 512:1024], pvb[0:65, :])
            nc.gpsimd.tensor_copy(sd[:], pvc[64:65, :])
            rc = norm.tile([1, 1024], f32, tag="rc")
            nc.vector.reciprocal_approx_fast(rc[:], sd[:])
            rb = norm.tile([64, 1024], f32, tag="rb")
            nc.gpsimd.partition_broadcast(rb[:], rc[:])
            for hh in range(2):
                meng = nc.vector if dvenorm else nc.gpsimd
                meng.tensor_mul(
                    ob[64 * hh:64 * hh + 64, pr, qt * 512:(qt + 1) * 512],
                    pvc[0:64, hh * 512:(hh + 1) * 512],
                    rb[:, hh * 512:(hh + 1) * 512])
            pump(4)

    # ---- schedule ----
    if attn_only:
        for qt in range(4):
            emit_attention(qt, rate=1.0)
        ot = work.tile([128, 512], bf16, tag="ot")
        nc.vector.tensor_copy(ot[:], ob[:, 0, 0:512])
        nc.sync.dma_start(out_d[0:128, 0:512], ot[:])
        return
    emit_qkv_first([0, 1, 2, 3])
    emit_qkv_first([4, 5, 6, 7])
    for tq in range(4):
        emit_v(tq)
    if qkv_only:
        for ec in range(8):
            enqueue_qkv_second(ec)
        for tq in range(4, 8):
            enqueue_v_second(tq)
        drain()
        ot = work.tile([128, 512], bf16, tag="ot")
        nc.vector.tensor_copy(ot[:], qk[:, 0, 0:512])
        nc.sync.dma_start(out_d[0:128, 0:512], ot[:])
        return

    for ec in range(4):
        enqueue_qkv_second(ec, deadline=(2, 0, 0))
    enqueue_qkv_second(4, deadline=(2, 0, 7))
    enqueue_v_second(4, deadline=(2, 0, 8))
    enqueue_v_second(5, deadline=(2, 0, 10))
    enqueue_qkv_second(5, deadline=(2, 1, 7))
    enqueue_qkv_second(6, deadline=(2, 2, 7))
    enqueue_qkv_second(7, deadline=(2, 3, 7))
    enqueue_v_second(6, deadline=(3, 0, 12))
    enqueue_v_second(7, deadline=(3, 0, 14))

    emit_attention(0, rate=1.5)
    enqueue_oproj(0)
    emit_attention(1, rate=1.5)
    enqueue_oproj(1)
    emit_attention(2, rate=1.2)
    enqueue_oproj(2, hold=3)
    emit_attention(3, rate=1.2)
    drain()
    # tail o_proj: 1-bank eh-split tiles throughout, pipelined via psa
    # rotation; copies alternate ACT/DVE, DMA alternates sync/gpsimd.
    tail_tts = list(reserve) + [12, 13, 14, 15]
    reserve.clear()
    for k, tt in enumerate(tail_tts):
        for eh in range(2):
            po = sp_tile(f"po3_{tt}e{eh}")
            for pr in range(4):
                nc.tensor.matmul(
                    po[:],
                    ob[:, pr, tt * 128:(tt + 1) * 128],
                    wos[pr][:, eh * 512:(eh + 1) * 512],
                    start=(pr == 0), stop=(pr == 3))
            ot = work.tile([128, 512], bf16, tag="ot")
            if (2 * k + eh) % 2 == 0:
                nc.scalar.activation(ot[:], po[:], Copy)
            else:
                nc.vector.tensor_copy(ot[:], po[:])
            q = nc.sync if (2 * k + eh) % 2 == 0 else nc.gpsimd
            q.dma_start(
                out_d[tt * 128:(tt + 1) * 128, eh * 512:(eh + 1) * 512],
                ot[:])


def _build(reps=1, opts=()):
    import concourse.mybir as mybir
    import concourse.tile as tile
    from concourse import bacc

    bf16 = mybir.dt.bfloat16
    f32 = mybir.dt.float32

    nc = bacc.Bacc("TRN2", target_bir_lowering=False, debug=False,
                   num_devices=NCORES)
    xt_d = nc.dram_tensor("xt", [8, 128, T], bf16, kind="ExternalInput")
    wq_d = nc.dram_tensor("wq", [8, 128, 1536], bf16, kind="ExternalInput")
    wo_d = nc.dram_tensor("wo", [4, 128, D], bf16, kind="ExternalInput")
    out_d = nc.dram_tensor("out", [T, D], bf16, kind="ExternalOutput")

    hsplit = 'hsplit' in opts
    with tile.TileContext(nc) as tc:
        with (
            tc.tile_pool(name="cst", bufs=1) as cst,
            tc.tile_pool(name="big", bufs=1) as big,
            tc.tile_pool(name="work", bufs=6) as work,
            tc.tile_pool(name="norm", bufs=2) as norm,
            tc.tile_pool(name="stp", bufs=6 if hsplit else 4) as stp,
            tc.tile_pool(name="psa", bufs=4 if hsplit else 2,
                         space="PSUM") as psa,
            tc.tile_pool(name="psb", bufs=1, space="PSUM") as psb,
            tc.tile_pool(name="psf", bufs=1, space="PSUM") as psf,
        ):
            # static causal mask for the 128x128 diagonal blocks, stored
            # twice so one strided mul covers both heads of a pair:
            # masks[p, h, q] = 1 if q >= p else 0
            masks = cst.tile([128, 2, 128], bf16)
            nc.gpsimd.memset(masks[:], 1.0)
            nc.gpsimd.affine_select(
                out=masks[:], in_=masks[:],
                compare_op=mybir.AluOpType.is_ge, fill=0.0,
                base=0, channel_multiplier=-1, pattern=[[0, 2], [1, 128]],
            )
            pools = (cst, big, work, norm, stp, psa, psb, psf)
            dram = (xt_d, wq_d, wo_d, out_d, masks)
            tc._kx_preloaded = {}
            if 'hoist_dma' in opts:
                if 'attn_only' in opts:
                    qk = big.tile([128, 8, T], mybir.dt.bfloat16, tag="qk")
                    vt = big.tile([128, NKC, HPC, DK + 1],
                                  mybir.dt.bfloat16, tag="vt")
                    nc.gpsimd.memset(qk[:], 0.02)
                    nc.gpsimd.memset(vt[:], 1.0)
                    tc._kx_preloaded['qkvt'] = (qk, vt)
                else:
                    tc._kx_preloaded['inputs'] = _emit_inputs(
                        nc, big, xt_d, wq_d, wo_d)
            emit = _emit_hsplit if hsplit else _emit
            if reps == 1:
                emit(nc, tc, pools, dram, opts)
            else:
                with tc.For_i(0, reps, 1):
                    emit(nc, tc, pools, dram, opts)

    nc.compile()
    return nc


def prep_inputs(x, w_qkv, w_o):
    """Host-side shard + layout prep. Returns in_maps for cores 0..7."""
    bf = ml_dtypes.bfloat16
    in_maps = []
    for c in range(NCORES):
        b, hh = c // 2, HPC * (c % 2)
        qrows = w_qkv[hh * DK:(hh + HPC) * DK]                    # [512, 1024]
        krows = w_qkv[D + hh * DK:D + (hh + HPC) * DK]
        vrows = w_qkv[2 * D + hh * DK:2 * D + (hh + HPC) * DK]
        wqt = np.concatenate([qrows, krows, vrows], 0).T          # [1024, 1536]
        in_maps.append({
            "xt": np.ascontiguousarray(x[b].T).astype(bf).reshape(8, 128, T),
            "wq": wqt.astype(bf).reshape(8, 128, 1536),
            "wo": np.ascontiguousarray(w_o[:, hh * DK:(hh + HPC) * DK].T)
                    .astype(bf).reshape(4, 128, D),
        })
    return in_maps


def get_nc(reps=1, opts=()):
    key = ("nc", reps, tuple(opts))
    if key not in _cache:
        _cache[key] = _build(reps, tuple(opts))
    return _cache[key]


DEFAULT_OPTS = ()


def kernel(x, w_qkv, w_o):
    from concourse.bass_utils import run_bass_kernel_spmd

    nc = get_nc(1, DEFAULT_OPTS)
    in_maps = prep_inputs(np.asarray(x, dtype=np.float32),
                          np.asarray(w_qkv, dtype=np.float32),
                          np.asarray(w_o, dtype=np.float32))
    try:
        res = run_bass_kernel_spmd(nc, in_maps, core_ids=list(range(NCORES)))
    except Exception:
        # transient device faults (e.g. NRT_EXEC_UNIT_UNRECOVERABLE) have
        # been observed once on an otherwise-correct build; retry once
        res = run_bass_kernel_spmd(nc, in_maps, core_ids=list(range(NCORES)))
    out = np.empty((B, T, D), np.float32)
    for b in range(B):
        out[b] = (res.results[2 * b]["out"].astype(np.float32)
                  + res.results[2 * b + 1]["out"].astype(np.float32))
    return out



# revision 23
# speedup vs baseline: 1.4626x; 1.4626x over previous
"""Causal self-attention (B=4, T=2048, D=1024, H=16) on 8 Trainium2 NeuronCores.

Sharding: batch x head-half. Core c handles batch b = c//2 and heads
hh..hh+7 where hh = 8*(c%2)  (tensor-parallel split of w_qkv output dim and
w_o input dim). Each core produces a partial o_proj output [2048, 1024] in
bf16; the host sums the two partials per batch in f32 (the 2-way
all-reduce).

Per-core kernel (all matmuls bf16, fp32 PSUM accumulate). Head pairs are
fused into [128, 1024] two-bank PSUM tiles throughout: phase 1 projects
Q^T,K^T head-pair-stacked [128, t] plus V in natural [t, dk] layout with an
appended ones column (rowsum trick, M=65 PV matmuls); phase 2 does causal
attention per (512-query tile, head pair) with S^T chunks, one strided exp
per chunk, static triangle masks on diagonal chunks; phase 3 o_proj.

Schedule (PE-saturating, sim ~233us vs ~283us for the phase-ordered
version):
  - input DMA interleaved by d-chunk across the SP and ACT queues; the
    first QKV phase runs dc-outer over 4 concurrent ec-group PSUMs so
    compute starts as soon as input chunk 0 lands.
  - the attention inner loop is software-pipelined (score kc+1 issued
    before PV kc) and a deadline-ordered filler queue weaves the
    second-half QKV / V / o_proj matmuls into the exp-bound attention
    stream so PE never idles; deadlines (qt, pr, kc) guarantee a filler
    group's output is emitted before the first attention chunk that
    reads it.
  - engine budget: exp on ACT; PSUM->SBUF copies on DVE (ACT during
    projection phases); triangle masks on DVE (HW A/B: Pool's strict
    per-Q7 FIFO made the exp->mask->PV chain ~100us slower); normalize
    muls and partition broadcast on Pool (no PSUM port, so everything it
    touches is SBUF); PSUM accumulators are freed early via a single
    [65,1024] evacuation copy per head pair; output DMA on SP, bf16.
  - exp instructions are flat contiguous [128,1024] for full-width
    chunks (lo==0); the strided 3-d AP form is only used for partial
    diagonal chunks. HW A/B: the strided exp cost ~35%/inst more on ACT
    and ~100us end-to-end.
  - PSUM budget (8 banks): 2x score tiles [128,1024] + 1 PV-pair tile
    [128,1024] + 2 half-bank filler tiles [128,512] with rotating tags.
"""
import numpy as np
import ml_dtypes

B, T, D, H = 4, 2048, 1024, 16
DK = D // H          # 64
HPC = 8              # heads per core
NCORES = 8
NQT = T // 512       # 4
NKC = T // 128       # 16

_cache = {}
FLAT_EXP = False


def _emit_inputs(nc, big, xt_d, wq_d, wo_d, skip_dma=False):
    """Input tiles + their DMA loads. d-chunk k (x half + qk-weight half)
    arrives ~in lockstep on two queues so the dc-outer first QKV phase is
    never input-starved. chunk 0 is further split in half so the very first
    matmul can start a few hundred ns earlier."""
    import concourse.mybir as mybir
    bf16 = mybir.dt.bfloat16

    xta = [big.tile([128, 1024], bf16, tag=f"xta{dc}", name=f"xta{dc}")
           for dc in range(8)]
    xtb = [big.tile([128, 1024], bf16, tag=f"xtb{dc}", name=f"xtb{dc}")
           for dc in range(8)]
    wqk = [big.tile([128, 1024], bf16, tag=f"wqk{dc}", name=f"wqk{dc}")
           for dc in range(8)]
    wv = [big.tile([128, 512], bf16, tag=f"wv{dc}", name=f"wv{dc}")
          for dc in range(8)]
    wos = [big.tile([128, D], bf16, tag=f"wo{pr}", name=f"wos{pr}")
           for pr in range(4)]
    if skip_dma:
        for dc in range(8):
            nc.gpsimd.memset(xta[dc][:], 0.01)
            nc.gpsimd.memset(xtb[dc][:], 0.01)
            nc.gpsimd.memset(wqk[dc][:], 0.01)
            nc.gpsimd.memset(wv[dc][:], 0.01)
        for pr in range(4):
            nc.gpsimd.memset(wos[pr][:], 0.01)
        return xta, xtb, wqk, wv, wos
    nc.gpsimd.dma_start(wqk[0][:, 0:512], wq_d[0, :, 0:512])
    for dc in range(8):
        if dc == 0:
            nc.sync.dma_start(xta[0][:, 0:512], xt_d[0, :, 0:512])
            nc.sync.dma_start(xta[0][:, 512:1024], xt_d[0, :, 512:1024])
            nc.scalar.dma_start(wqk[0][:, 512:1024], wq_d[0, :, 512:1024])
            continue
        nc.sync.dma_start(xta[dc][:], xt_d[dc, :, 0:1024])
        nc.scalar.dma_start(wqk[dc][:], wq_d[dc, :, 0:1024])
    for dc in range(8):
        nc.sync.dma_start(xtb[dc][:], xt_d[dc, :, 1024:2048])
        nc.scalar.dma_start(wv[dc][:], wq_d[dc, :, 1024:1536])
    for pr in range(4):
        nc.scalar.dma_start(wos[pr][:], wo_d[pr])
    return xta, xtb, wqk, wv, wos


def _emit(nc, tc, pools, dram, opts=()):
    import concourse.mybir as mybir

    bf16 = mybir.dt.bfloat16
    f32 = mybir.dt.float32
    Exp = mybir.ActivationFunctionType.Exp
    Copy = mybir.ActivationFunctionType.Copy
    cst, big, work, norm, stp, psa, psb, psf = pools
    xt_d, wq_d, wo_d, out_d, masks = dram
    flat_exp = FLAT_EXP or ('flat_exp' in opts)
    nomask = 'nomask' in opts
    nonorm = 'nonorm' in opts
    nopv = 'nopv' in opts
    # defaults chosen by HW A/B (same-batch): flat exp for full-width chunks
    # and the diagonal mask on DVE instead of Pool are each ~100us faster.
    oneflat = 'no_oneflat' not in opts
    dvemask = 'no_dvemask' not in opts
    mask2 = 'mask2' in opts
    dvemask2 = 'dvemask2' in opts
    # normalize ob muls on DVE: -121us vs Pool (HW A/B, same batch)
    dvenorm = 'no_dvenorm' not in opts
    dvesd = 'dvesd' in opts
    actsd = 'actsd' in opts
    if 'rlow' in opts:
        rates = (1.2, 1.2, 1.0, 1.0)
    elif 'rhigh' in opts:
        rates = (1.8, 1.8, 1.5, 1.5)
    else:
        rates = (1.5, 1.5, 1.2, 1.2)

    qkv_only = 'qkv_only' in opts
    attn_only = 'attn_only' in opts

    pre = getattr(tc, '_kx_preloaded', {})
    if attn_only:
        if 'qkvt' in pre:
            qk, vt = pre['qkvt']
        else:
            qk = big.tile([128, 8, T], bf16, tag="qk")
            vt = big.tile([128, NKC, HPC, DK + 1], bf16, tag="vt")
            nc.gpsimd.memset(qk[:], 0.02)
            nc.gpsimd.memset(vt[:], 1.0)
        ob = big.tile([128, 4, T], bf16, tag="ob")
        xta = xtb = wqk = wv = wos = None
    else:
        if 'inputs' in pre:
            xta, xtb, wqk, wv, wos = pre['inputs']
        else:
            xta, xtb, wqk, wv, wos = _emit_inputs(nc, big, xt_d, wq_d, wo_d)
        qk = big.tile([128, 8, T], bf16, tag="qk")
        vt = big.tile([128, NKC, HPC, DK + 1], bf16, tag="vt")
        ob = big.tile([128, 4, T], bf16, tag="ob")
        nc.gpsimd.memset(vt[:, :, :, DK], 1.0)


    # ---- filler machinery ----------------------------------------------
    # Filler groups are sequences of ~1-matmul closures writing one
    # [128,512] PSUM bank allocated from two rotating tags (filA/filB) so
    # consecutive groups double-buffer. Groups carry a deadline
    # (qt, pr, kc): the attention chunk whose emission needs their output;
    # force() drains all due groups, pump() feeds them in at a given rate
    # to cover PE idle while ACT runs the exps. The queue is kept sorted
    # by deadline (None = last) and only whole groups are ever reordered.
    groups = []        # list of [deadline, [closures...]]
    reserve = []
    pump_acc = [0.0]
    tag_rot = [0]

    def fil_tile(name):
        tag = ("filA", "filB")[tag_rot[0] & 1]
        tag_rot[0] += 1
        return psf.tile([128, 512], f32, tag=tag, name=name)

    def enqueue(deadline, closures):
        i = len(groups)
        if deadline is not None:
            while i > 0 and (groups[i - 1][0] is None
                             or groups[i - 1][0] > deadline):
                i -= 1
        groups.insert(i, [deadline, list(closures)])

    def _pop_one():
        g = groups[0]
        g[1].pop(0)()
        if not g[1]:
            groups.pop(0)

    def pump(n=1.0):
        pump_acc[0] += n
        while pump_acc[0] >= 1.0 and groups:
            _pop_one()
            pump_acc[0] -= 1.0

    def force(key):
        while groups and groups[0][0] is not None and groups[0][0] <= key:
            _pop_one()

    def drain():
        while groups:
            _pop_one()

    # ---- projection emitters -------------------------------------------
    # first-half QKV (t 0:1024): dc-outer over 4 live ec-groups so compute
    # starts as soon as input chunk 0 lands. ec-groups 0..2 use the three
    # [128,1024] PSUM tiles; ec-group 3 uses the two half-bank filler tiles.
    def emit_qkv_first(ec_group):
        t01 = psa.tile([128, 1024], f32, tag="s", name=f"qA{ec_group[0]}")
        t1 = psa.tile([128, 1024], f32, tag="s", name=f"qA{ec_group[1]}")
        t2 = psb.tile([128, 1024], f32, tag="pv", name=f"qA{ec_group[2]}")
        t3a = fil_tile(f"qA{ec_group[3]}a")
        t3b = fil_tile(f"qA{ec_group[3]}b")
        views = {ec_group[0]: (t01[:, 0:512], t01[:, 512:1024]),
                 ec_group[1]: (t1[:, 0:512], t1[:, 512:1024]),
                 ec_group[2]: (t2[:, 0:512], t2[:, 512:1024]),
                 ec_group[3]: (t3a[:], t3b[:])}
        for dc in range(8):
            for ec in ec_group:
                for half in range(2):
                    nc.tensor.matmul(
                        views[ec][half],
                        wqk[dc][:, ec * 128:(ec + 1) * 128],
                        xta[dc][:, half * 512:(half + 1) * 512],
                        start=(dc == 0), stop=(dc == 7),
                    )
        for gi, ec in enumerate(ec_group):
            if gi < 3:
                src = (t01, t1, t2)[gi]
                nc.scalar.activation(qk[:, ec, 0:1024], src[:], Copy)
            else:
                nc.scalar.activation(qk[:, ec, 0:512], t3a[:], Copy)
                nc.scalar.activation(qk[:, ec, 512:1024], t3b[:], Copy)

    # V projection for t rows [256*tq, 256*tq+256), first half (phase)
    def emit_v(tq):
        ps = psa.tile([128, 1024], f32, tag="s", name=f"vps{tq}")
        for half in range(2):
            tt = 2 * tq + half
            for dc in range(8):
                nc.tensor.matmul(
                    ps[:, half * 512:(half + 1) * 512],
                    xta[dc][:, (tt % 8) * 128:(tt % 8) * 128 + 128],
                    wv[dc][:],
                    start=(dc == 0), stop=(dc == 7),
                )
        nc.scalar.activation(
            vt[:, 2 * tq:2 * tq + 2, :, 0:DK],
            ps[:].rearrange("p (t h d) -> p t h d", t=2, d=DK), Copy)

    # ---- filler group builders (second halves + o_proj) -----------------
    def enqueue_qkv_second(ec, deadline=None):
        for half in range(2):
            box = {}

            def mk(dc, half=half, box=box):
                def f():
                    if dc == 0:
                        box["ps"] = fil_tile(f"qB{ec}h{half}")
                    nc.tensor.matmul(
                        box["ps"][:],
                        wqk[dc][:, ec * 128:(ec + 1) * 128],
                        xtb[dc][:, half * 512:(half + 1) * 512],
                        start=(dc == 0), stop=(dc == 7),
                    )
                    if dc == 7:
                        nc.vector.tensor_copy(
                            qk[:, ec, 1024 + half * 512:1536 + half * 512],
                            box["ps"][:])
                return f
            enqueue(deadline, [mk(dc) for dc in range(8)])

    def enqueue_v_second(tq, deadline=None):
        for half in range(2):
            tt = 2 * tq + half
            box = {}

            def mk(dc, tt=tt, box=box):
                def f():
                    if dc == 0:
                        box["ps"] = fil_tile(f"vB{tt}")
                    nc.tensor.matmul(
                        box["ps"][:],
                        xtb[dc][:, (tt % 8) * 128:(tt % 8) * 128 + 128],
                        wv[dc][:],
                        start=(dc == 0), stop=(dc == 7),
                    )
                    if dc == 7:
                        nc.vector.tensor_copy(
                            vt[:, tt, :, 0:DK],
                            box["ps"][:].rearrange("p (h d) -> p h d", d=DK))
                return f
            enqueue(deadline, [mk(dc) for dc in range(8)])

    def oproj_half_closures(tt, eh, act_copy=False):
        box = {}

        def mk(pr):
            def f():
                if pr == 0:
                    box["po"] = fil_tile(f"po{tt}e{eh}")
                nc.tensor.matmul(
                    box["po"][:],
                    ob[:, pr, tt * 128:(tt + 1) * 128],
                    wos[pr][:, eh * 512:(eh + 1) * 512],
                    start=(pr == 0), stop=(pr == 3))
                if pr == 3:
                    ot = work.tile([128, 512], bf16, tag="ot")
                    if act_copy:
                        nc.scalar.activation(ot[:], box["po"][:], Copy)
                    else:
                        nc.vector.tensor_copy(ot[:], box["po"][:])
                    nc.sync.dma_start(
                        out_d[tt * 128:(tt + 1) * 128,
                              eh * 512:(eh + 1) * 512], ot[:])
            return f
        return [mk(pr) for pr in range(4)]

    def enqueue_oproj(qt, hold=0):
        for tt in range(4 * qt, 4 * qt + 4):
            if hold and tt >= 4 * qt + 4 - hold:
                reserve.append(tt)
                continue
            for eh in range(2):
                enqueue(None, oproj_half_closures(tt, eh))

    # ---- attention for one 512-query tile, software-pipelined -----------
    def emit_attention(qt, rate=1.0):
        nkc = 4 * qt + 4
        for pr in range(4):          # head pair (2*pr, 2*pr+1)
            pvp = psb.tile([128, 1024], f32, tag="pv")
            sts = {}

            def emit_score(kc):
                i = kc - 4 * qt
                lo = max(i, 0) * 128
                sp = psa.tile([128, 1024], f32, tag="s")
                nc.tensor.matmul(
                    sp[:, lo:512], qk[0:64, 4 + pr, kc * 128:(kc + 1) * 128],
                    qk[0:64, pr, qt * 512 + lo:(qt + 1) * 512],
                    start=True, stop=True, tile_position=(0, 0))
                nc.tensor.matmul(
                    sp[:, 512 + lo:1024],
                    qk[64:128, 4 + pr, kc * 128:(kc + 1) * 128],
                    qk[64:128, pr, qt * 512 + lo:(qt + 1) * 512],
                    start=True, stop=True, tile_position=(64, 0))
                st = stp.tile([128, 1024], bf16, tag="st")
                if flat_exp:
                    nc.scalar.activation(st[:, lo:512], sp[:, lo:512],
                                         Exp, scale=0.125)
                    nc.scalar.activation(st[:, 512 + lo:1024],
                                         sp[:, 512 + lo:1024],
                                         Exp, scale=0.125)
                elif oneflat and lo == 0:
                    # full-width chunk: both head halves span their whole 512
                    # columns, so one flat contiguous [128,1024] exp (the
                    # strided 3-d AP costs ~35% more on ACT and was the
                    # dominant drag on the attention stream).
                    nc.scalar.activation(st[:], sp[:], Exp, scale=0.125)
                else:
                    sp3 = sp[:].rearrange("p (h q) -> p h q", h=2)[:, :, lo:]
                    st3 = st[:].rearrange("p (h q) -> p h q", h=2)[:, :, lo:]
                    nc.scalar.activation(st3, sp3, Exp, scale=0.125)
                if i >= 0 and not nomask:
                    # diagonal chunk: mask the triangular blocks
                    if mask2 or dvemask2:
                        # two contiguous [128,128] muls (no strided AP)
                        eng = nc.vector if dvemask2 else nc.gpsimd
                        for hh in range(2):
                            blk = st[:, hh * 512 + lo:hh * 512 + lo + 128]
                            eng.tensor_mul(blk, blk, masks[:, hh, :])
                    else:
                        std = st[:].rearrange(
                            "p (h q) -> p h q", h=2)[:, :, lo:lo + 128]
                        eng = nc.vector if dvemask else nc.gpsimd
                        eng.tensor_mul(std, std, masks[:])
                sts[kc] = st

            def emit_pv(kc):
                i = kc - 4 * qt
                lo = max(i, 0) * 128
                st = sts.pop(kc)
                if nopv:
                    return
                nc.tensor.matmul(
                    pvp[0:65, lo:512], vt[:, kc, 2 * pr, :], st[:, lo:512],
                    start=(kc == 0), stop=(kc == nkc - 1))
                nc.tensor.matmul(
                    pvp[0:65, 512 + lo:1024], vt[:, kc, 2 * pr + 1, :],
                    st[:, 512 + lo:1024],
                    start=(kc == 0), stop=(kc == nkc - 1))

            force((qt, pr, 0))
            emit_score(0)
            for kc in range(nkc):
                force((qt, pr, kc))
                if kc + 1 < nkc:
                    emit_score(kc + 1)
                pump(rate)
                emit_pv(kc)

            # normalize: evacuate pvp in one copy (frees the PSUM bank pair
            # fast), then 1/rowsum on the SBUF copy. Pool has no PSUM port,
            # so: DVE evac -> Pool sd stage -> DVE recip -> Pool broadcast
            # -> Pool muls.
            if nonorm or nopv:
                pump(4)
                continue
            pvc = work.tile([65, 1024], f32, tag="pvc")
            sd = norm.tile([1, 1024], f32, tag="sd")
            if qt == 3 and pr == 3:
                nc.scalar.activation(sd[:], pvp[64:65, :], Copy)
                nc.vector.tensor_copy(pvc[:], pvp[0:65, :])
            else:
                nc.vector.tensor_copy(pvc[:], pvp[0:65, :])
                if dvesd:
                    nc.vector.tensor_copy(sd[:], pvc[64:65, :])
                elif actsd:
                    nc.scalar.activation(sd[:], pvc[64:65, :], Copy)
                else:
                    nc.gpsimd.tensor_copy(sd[:], pvc[64:65, :])
            rc = norm.tile([1, 1024], f32, tag="rc")
            nc.vector.reciprocal_approx_fast(rc[:], sd[:])
            rb = norm.tile([64, 1024], f32, tag="rb")
            nc.gpsimd.partition_broadcast(rb[:], rc[:])
            for hh in range(2):
                meng = nc.vector if dvenorm else nc.gpsimd
                meng.tensor_mul(
                    ob[64 * hh:64 * hh + 64, pr, qt * 512:(qt + 1) * 512],
                    pvc[0:64, hh * 512:(hh + 1) * 512],
                    rb[:, hh * 512:(hh + 1) * 512])
            pump(4)

    # ---- schedule ----
    if attn_only:
        for qt in range(4):
            emit_attention(qt, rate=1.0)
        ot = work.tile([128, 512], bf16, tag="ot")
        nc.vector.tensor_copy(ot[:], ob[:, 0, 0:512])
        nc.sync.dma_start(out_d[0:128, 0:512], ot[:])
        return
    emit_qkv_first([0, 1, 2, 3])     # Q heads 0..7, t 0:1024
    emit_qkv_first([4, 5, 6, 7])     # K heads 0..7, t 0:1024
    for tq in range(4):              # V rows 0:1024
        emit_v(tq)
    if qkv_only:
        for ec in range(8):
            enqueue_qkv_second(ec)
        for tq in range(4, 8):
            enqueue_v_second(tq)
        drain()
        ot = work.tile([128, 512], bf16, tag="ot")
        nc.vector.tensor_copy(ot[:], qk[:, 0, 0:512])
        nc.sync.dma_start(out_d[0:128, 0:512], ot[:])
        return

    # Q cols 1024:1536 must exist when attention(2) starts; K pair p's
    # second half is first read at attention(2) pair p chunk 8; vt rows
    # 8:12 at attention(2) pair 0 chunks 8/10; vt rows 12:16 at
    # attention(3) pair 0 chunks 12/14. Everything else floats free.
    for ec in range(4):
        enqueue_qkv_second(ec, deadline=(2, 0, 0))
    enqueue_qkv_second(4, deadline=(2, 0, 7))
    enqueue_v_second(4, deadline=(2, 0, 8))
    enqueue_v_second(5, deadline=(2, 0, 10))
    enqueue_qkv_second(5, deadline=(2, 1, 7))
    enqueue_qkv_second(6, deadline=(2, 2, 7))
    enqueue_qkv_second(7, deadline=(2, 3, 7))
    enqueue_v_second(6, deadline=(3, 0, 12))
    enqueue_v_second(7, deadline=(3, 0, 14))

    emit_attention(0, rate=rates[0])
    enqueue_oproj(0)
    emit_attention(1, rate=rates[1])
    enqueue_oproj(1)
    emit_attention(2, rate=rates[2])
    enqueue_oproj(2, hold=3)
    emit_attention(3, rate=rates[3])
    drain()
    # tail o_proj (held tiles + qt=3): full-width PSUM tiles from the
    # now-free big pools -- no dependency on the half-bank filler slots,
    # whose recycling is gated by laggy DVE copies. Copies alternate
    # ACT/DVE so they pipeline; psb tiles come later so the last head
    # pair's PSUM evacuation has finished.
    tail_tts = list(reserve) + [12, 13, 14, 15]
    reserve.clear()
    tail_pools = [(psa, "s"), (psa, "s"), (psb, "pv")]
    for k, tt in enumerate(tail_tts):
        if k >= len(tail_tts) - 2:
            # last two tiles: eh-split into half-bank PSUMs so each
            # evacuation copy starts four matmuls earlier
            for eh in range(2):
                po = fil_tile(f"po3_{tt}e{eh}")
                for pr in range(4):
                    nc.tensor.matmul(
                        po[:],
                        ob[:, pr, tt * 128:(tt + 1) * 128],
                        wos[pr][:, eh * 512:(eh + 1) * 512],
                        start=(pr == 0), stop=(pr == 3))
                ot = work.tile([128, 512], bf16, tag="ot")
                if eh == 0:
                    nc.scalar.activation(ot[:], po[:], Copy)
                else:
                    nc.vector.tensor_copy(ot[:], po[:])
                q = nc.sync if eh == 0 else nc.gpsimd
                q.dma_start(
                    out_d[tt * 128:(tt + 1) * 128, eh * 512:(eh + 1) * 512],
                    ot[:])
            continue
        pool, tag = tail_pools[k % 3]
        po = pool.tile([128, 1024], f32, tag=tag, name=f"po3_{tt}")
        for eh in range(2):
            for pr in range(4):
                nc.tensor.matmul(
                    po[:, eh * 512:(eh + 1) * 512],
                    ob[:, pr, tt * 128:(tt + 1) * 128],
                    wos[pr][:, eh * 512:(eh + 1) * 512],
                    start=(pr == 0), stop=(pr == 3))
        for eh in range(2):
            ot = work.tile([128, 512], bf16, tag="ot")
            if k % 2 == 0:
                nc.scalar.activation(ot[:], po[:, eh * 512:(eh + 1) * 512],
                                     Copy)
            else:
                nc.vector.tensor_copy(ot[:], po[:, eh * 512:(eh + 1) * 512])
            q = nc.sync if (2 * k + eh) % 2 == 0 else nc.gpsimd
            q.dma_start(
                out_d[tt * 128:(tt + 1) * 128, eh * 512:(eh + 1) * 512],
                ot[:])


def _emit_hsplit(nc, tc, pools, dram, opts=()):
    """Head-split attention: per-head 1-bank score/PV PSUM tiles so the two
    heads of a pair form independent score->exp->PV streams; cross-engine
    semaphore latency hides under the sibling stream. PSUM: 4x sp [128,512]
    (psa) + 2x pvp [128,512] (psb pva/pvb) + 2x filler (psf) = 8 banks."""
    import concourse.mybir as mybir

    bf16 = mybir.dt.bfloat16
    f32 = mybir.dt.float32
    Exp = mybir.ActivationFunctionType.Exp
    Copy = mybir.ActivationFunctionType.Copy
    cst, big, work, norm, stp, psa, psb, psf = pools
    xt_d, wq_d, wo_d, out_d, masks = dram
    qkv_only = 'qkv_only' in opts
    attn_only = 'attn_only' in opts

    pre = getattr(tc, '_kx_preloaded', {})
    if attn_only:
        if 'qkvt' in pre:
            qk, vt = pre['qkvt']
        else:
            qk = big.tile([128, 8, T], bf16, tag="qk")
            vt = big.tile([128, NKC, HPC, DK + 1], bf16, tag="vt")
            nc.gpsimd.memset(qk[:], 0.02)
            nc.gpsimd.memset(vt[:], 1.0)
        ob = big.tile([128, 4, T], bf16, tag="ob")
        xta = xtb = wqk = wv = wos = None
    else:
        if 'inputs' in pre:
            xta, xtb, wqk, wv, wos = pre['inputs']
        else:
            xta, xtb, wqk, wv, wos = _emit_inputs(nc, big, xt_d, wq_d, wo_d)
        qk = big.tile([128, 8, T], bf16, tag="qk")
        vt = big.tile([128, NKC, HPC, DK + 1], bf16, tag="vt")
        ob = big.tile([128, 4, T], bf16, tag="ob")
        nc.gpsimd.memset(vt[:, :, :, DK], 1.0)

    # ---- filler machinery (identical to _emit) -------------------------
    groups = []
    reserve = []
    pump_acc = [0.0]
    tag_rot = [0]

    def fil_tile(name):
        tag = ("filA", "filB")[tag_rot[0] & 1]
        tag_rot[0] += 1
        return psf.tile([128, 512], f32, tag=tag, name=name)

    def enqueue(deadline, closures):
        i = len(groups)
        if deadline is not None:
            while i > 0 and (groups[i - 1][0] is None
                             or groups[i - 1][0] > deadline):
                i -= 1
        groups.insert(i, [deadline, list(closures)])

    def _pop_one():
        g = groups[0]
        g[1].pop(0)()
        if not g[1]:
            groups.pop(0)

    def pump(n=1.0):
        pump_acc[0] += n
        while pump_acc[0] >= 1.0 and groups:
            _pop_one()
            pump_acc[0] -= 1.0

    def force(key):
        while groups and groups[0][0] is not None and groups[0][0] <= key:
            _pop_one()

    def drain():
        while groups:
            _pop_one()

    def sp_tile(name):
        return psa.tile([128, 512], f32, tag="s", name=name)

    # ---- projection emitters (1-bank tiles) ----------------------------
    def emit_qkv_first(ec_group):
        slots = [sp_tile(f"qA{ec_group[0]}a"), sp_tile(f"qA{ec_group[0]}b"),
                 sp_tile(f"qA{ec_group[1]}a"), sp_tile(f"qA{ec_group[1]}b"),
                 psb.tile([128, 512], f32, tag="pva", name=f"qA{ec_group[2]}a"),
                 psb.tile([128, 512], f32, tag="pvb", name=f"qA{ec_group[2]}b"),
                 fil_tile(f"qA{ec_group[3]}a"), fil_tile(f"qA{ec_group[3]}b")]
        for dc in range(8):
            for gi, ec in enumerate(ec_group):
                for half in range(2):
                    nc.tensor.matmul(
                        slots[2 * gi + half][:],
                        wqk[dc][:, ec * 128:(ec + 1) * 128],
                        xta[dc][:, half * 512:(half + 1) * 512],
                        start=(dc == 0), stop=(dc == 7),
                    )
        for gi, ec in enumerate(ec_group):
            for half in range(2):
                dst = qk[:, ec, half * 512:(half + 1) * 512]
                src = slots[2 * gi + half][:]
                if gi % 2 == 0:
                    nc.scalar.activation(dst, src, Copy)
                else:
                    nc.vector.tensor_copy(dst, src)

    def emit_v(tq):
        for half in range(2):
            tt = 2 * tq + half
            ps = sp_tile(f"vps{tt}")
            for dc in range(8):
                nc.tensor.matmul(
                    ps[:],
                    xta[dc][:, (tt % 8) * 128:(tt % 8) * 128 + 128],
                    wv[dc][:],
                    start=(dc == 0), stop=(dc == 7),
                )
            nc.scalar.activation(
                vt[:, tt, :, 0:DK],
                ps[:].rearrange("p (h d) -> p h d", d=DK), Copy)

    # ---- filler group builders (same as _emit) -------------------------
    def enqueue_qkv_second(ec, deadline=None):
        for half in range(2):
            box = {}

            def mk(dc, half=half, box=box):
                def f():
                    if dc == 0:
                        box["ps"] = fil_tile(f"qB{ec}h{half}")
                    nc.tensor.matmul(
                        box["ps"][:],
                        wqk[dc][:, ec * 128:(ec + 1) * 128],
                        xtb[dc][:, half * 512:(half + 1) * 512],
                        start=(dc == 0), stop=(dc == 7),
                    )
                    if dc == 7:
                        nc.vector.tensor_copy(
                            qk[:, ec, 1024 + half * 512:1536 + half * 512],
                            box["ps"][:])
                return f
            enqueue(deadline, [mk(dc) for dc in range(8)])

    def enqueue_v_second(tq, deadline=None):
        for half in range(2):
            tt = 2 * tq + half
            box = {}

            def mk(dc, tt=tt, box=box):
                def f():
                    if dc == 0:
                        box["ps"] = fil_tile(f"vB{tt}")
                    nc.tensor.matmul(
                        box["ps"][:],
                        xtb[dc][:, (tt % 8) * 128:(tt % 8) * 128 + 128],
                        wv[dc][:],
                        start=(dc == 0), stop=(dc == 7),
                    )
                    if dc == 7:
                        nc.vector.tensor_copy(
                            vt[:, tt, :, 0:DK],
                            box["ps"][:].rearrange("p (h d) -> p h d", d=DK))
                return f
            enqueue(deadline, [mk(dc) for dc in range(8)])

    def oproj_half_closures(tt, eh, act_copy=False):
        box = {}

        def mk(pr):
            def f():
                if pr == 0:
                    box["po"] = fil_tile(f"po{tt}e{eh}")
                nc.tensor.matmul(
                    box["po"][:],
                    ob[:, pr, tt * 128:(tt + 1) * 128],
                    wos[pr][:, eh * 512:(eh + 1) * 512],
                    start=(pr == 0), stop=(pr == 3))
                if pr == 3:
                    ot = work.tile([128, 512], bf16, tag="ot")
                    if act_copy:
                        nc.scalar.activation(ot[:], box["po"][:], Copy)
                    else:
                        nc.vector.tensor_copy(ot[:], box["po"][:])
                    nc.sync.dma_start(
                        out_d[tt * 128:(tt + 1) * 128,
                              eh * 512:(eh + 1) * 512], ot[:])
            return f
        return [mk(pr) for pr in range(4)]

    def enqueue_oproj(qt, hold=0):
        for tt in range(4 * qt, 4 * qt + 4):
            if hold and tt >= 4 * qt + 4 - hold:
                reserve.append(tt)
                continue
            for eh in range(2):
                enqueue(None, oproj_half_closures(tt, eh))

    mask2d = masks[:, 0, :]

    # ---- attention: head-split streams ---------------------------------
    def emit_attention(qt, rate=1.0):
        nkc = 4 * qt + 4
        for pr in range(4):
            pva = psb.tile([128, 512], f32, tag="pva", name=f"pva{qt}_{pr}")
            pvb = psb.tile([128, 512], f32, tag="pvb", name=f"pvb{qt}_{pr}")
            sts = {}

            def emit_score(kc, pva=pva, pvb=pvb, sts=sts):
                i = kc - 4 * qt
                lo = max(i, 0) * 128
                spa = sp_tile(f"sa{qt}{pr}_{kc}")
                spb = sp_tile(f"sb{qt}{pr}_{kc}")
                nc.tensor.matmul(
                    spa[:, lo:512], qk[0:64, 4 + pr, kc * 128:(kc + 1) * 128],
                    qk[0:64, pr, qt * 512 + lo:(qt + 1) * 512],
                    start=True, stop=True, tile_position=(0, 0))
                nc.tensor.matmul(
                    spb[:, lo:512],
                    qk[64:128, 4 + pr, kc * 128:(kc + 1) * 128],
                    qk[64:128, pr, qt * 512 + lo:(qt + 1) * 512],
                    start=True, stop=True, tile_position=(64, 0))
                sta = stp.tile([128, 512], bf16, tag="st", name=f"ta{kc}")
                stb = stp.tile([128, 512], bf16, tag="st", name=f"tb{kc}")
                nc.scalar.activation(sta[:, lo:512], spa[:, lo:512], Exp,
                                     scale=0.125)
                nc.scalar.activation(stb[:, lo:512], spb[:, lo:512], Exp,
                                     scale=0.125)
                if i >= 0:
                    nc.gpsimd.tensor_mul(sta[:, lo:lo + 128],
                                         sta[:, lo:lo + 128], mask2d)
                    nc.gpsimd.tensor_mul(stb[:, lo:lo + 128],
                                         stb[:, lo:lo + 128], mask2d)
                sts[kc] = (sta, stb)

            def emit_pv(kc, pva=pva, pvb=pvb, sts=sts):
                i = kc - 4 * qt
                lo = max(i, 0) * 128
                sta, stb = sts.pop(kc)
                nc.tensor.matmul(
                    pva[0:65, lo:512], vt[:, kc, 2 * pr, :], sta[:, lo:512],
                    start=(kc == 0), stop=(kc == nkc - 1))
                nc.tensor.matmul(
                    pvb[0:65, lo:512], vt[:, kc, 2 * pr + 1, :],
                    stb[:, lo:512],
                    start=(kc == 0), stop=(kc == nkc - 1))

            force((qt, pr, 0))
            emit_score(0)
            for kc in range(nkc):
                force((qt, pr, kc))
                if kc + 1 < nkc:
                    emit_score(kc + 1)
                pump(rate)
                emit_pv(kc)

            pvc = work.tile([65, 1024], f32, tag="pvc")
            sd = norm.tile([1, 1024], f32, tag="sd")
            nc.vector.tensor_copy(pvc[:, 0:512], pva[0:65, :])
            nc.vector.tensor_copy(pvc[:, 512:1024], pvb[0:65, :])
            nc.gpsimd.tensor_copy(sd[:], pvc[64:65, :])
            rc = norm.tile([1, 1024], f32, tag="rc")
            nc.vector.reciprocal_approx_fast(rc[:], sd[:])
            rb = norm.tile([64, 1024], f32, tag="rb")
            nc.gpsimd.partition_broadcast(rb[:], rc[:])
            for hh in range(2):
                meng = nc.vector if dvenorm else nc.gpsimd
                meng.tensor_mul(
                    ob[64 * hh:64 * hh + 64, pr, qt * 512:(qt + 1) * 512],
                    pvc[0:64, hh * 512:(hh + 1) * 512],
                    rb[:, hh * 512:(hh + 1) * 512])
            pump(4)

    # ---- schedule ----
    if attn_only:
        for qt in range(4):
            emit_attention(qt, rate=1.0)
        ot = work.tile([128, 512], bf16, tag="ot")
        nc.vector.tensor_copy(ot[:], ob[:, 0, 0:512])
        nc.sync.dma_start(out_d[0:128, 0:512], ot[:])
        return
    emit_qkv_first([0, 1, 2, 3])
    emit_qkv_first([4, 5, 6, 7])
    for tq in range(4):
        emit_v(tq)
    if qkv_only:
        for ec in range(8):
            enqueue_qkv_second(ec)
        for tq in range(4, 8):
            enqueue_v_second(tq)
        drain()
        ot = work.tile([128, 512], bf16, tag="ot")
        nc.vector.tensor_copy(ot[:], qk[:, 0, 0:512])
        nc.sync.dma_start(out_d[0:128, 0:512], ot[:])
        return

    for ec in range(4):
        enqueue_qkv_second(ec, deadline=(2, 0, 0))
    enqueue_qkv_second(4, deadline=(2, 0, 7))
    enqueue_v_second(4, deadline=(2, 0, 8))
    enqueue_v_second(5, deadline=(2, 0, 10))
    enqueue_qkv_second(5, deadline=(2, 1, 7))
    enqueue_qkv_second(6, deadline=(2, 2, 7))
    enqueue_qkv_second(7, deadline=(2, 3, 7))
    enqueue_v_second(6, deadline=(3, 0, 12))
    enqueue_v_second(7, deadline=(3, 0, 14))

    emit_attention(0, rate=1.5)
    enqueue_oproj(0)
    emit_attention(1, rate=1.5)
    enqueue_oproj(1)
    emit_attention(2, rate=1.2)
    enqueue_oproj(2, hold=3)
    emit_attention(3, rate=1.2)
    drain()
    # tail o_proj: 1-bank eh-split tiles throughout, pipelined via psa
    # rotation; copies alternate ACT/DVE, DMA alternates sync/gpsimd.
    tail_tts = list(reserve) + [12, 13, 14, 15]
    reserve.clear()
    for k, tt in enumerate(tail_tts):
        for eh in range(2):
            po = sp_tile(f"po3_{tt}e{eh}")
            for pr in range(4):
                nc.tensor.matmul(
                    po[:],
                    ob[:, pr, tt * 128:(tt + 1) * 128],
                    wos[pr][:, eh * 512:(eh + 1) * 512],
                    start=(pr == 0), stop=(pr == 3))
            ot = work.tile([128, 512], bf16, tag="ot")
            if (2 * k + eh) % 2 == 0:
                nc.scalar.activation(ot[:], po[:], Copy)
            else:
                nc.vector.tensor_copy(ot[:], po[:])
            q = nc.sync if (2 * k + eh) % 2 == 0 else nc.gpsimd
            q.dma_start(
                out_d[tt * 128:(tt + 1) * 128, eh * 512:(eh + 1) * 512],
                ot[:])


def _build(reps=1, opts=()):
    import concourse.mybir as mybir
    import concourse.tile as tile
    from concourse import bacc

    bf16 = mybir.dt.bfloat16
    f32 = mybir.dt.float32

    nc = bacc.Bacc("TRN2", target_bir_lowering=False, debug=False,
                   num_devices=NCORES)
    xt_d = nc.dram_tensor("xt", [8, 128, T], bf16, kind="ExternalInput")
    wq_d = nc.dram_tensor("wq", [8, 128, 1536], bf16, kind="ExternalInput")
    wo_d = nc.dram_tensor("wo", [4, 128, D], bf16, kind="ExternalInput")
    out_d = nc.dram_tensor("out", [T, D], bf16, kind="ExternalOutput")

    hsplit = 'hsplit' in opts
    with tile.TileContext(nc) as tc:
        with (
            tc.tile_pool(name="cst", bufs=1) as cst,
            tc.tile_pool(name="big", bufs=1) as big,
            tc.tile_pool(name="work", bufs=6) as work,
            tc.tile_pool(name="norm", bufs=2) as norm,
            tc.tile_pool(name="stp", bufs=6 if hsplit else 4) as stp,
            tc.tile_pool(name="psa", bufs=4 if hsplit else 2,
                         space="PSUM") as psa,
            tc.tile_pool(name="psb", bufs=1, space="PSUM") as psb,
            tc.tile_pool(name="psf", bufs=1, space="PSUM") as psf,
        ):
            # static causal mask for the 128x128 diagonal blocks, stored
            # twice so one strided mul covers both heads of a pair:
            # masks[p, h, q] = 1 if q >= p else 0
            masks = cst.tile([128, 2, 128], bf16)
            nc.gpsimd.memset(masks[:], 1.0)
            nc.gpsimd.affine_select(
                out=masks[:], in_=masks[:],
                compare_op=mybir.AluOpType.is_ge, fill=0.0,
                base=0, channel_multiplier=-1, pattern=[[0, 2], [1, 128]],
            )
            pools = (cst, big, work, norm, stp, psa, psb, psf)
            dram = (xt_d, wq_d, wo_d, out_d, masks)
            tc._kx_preloaded = {}
            if 'hoist_dma' in opts:
                if 'attn_only' in opts:
                    qk = big.tile([128, 8, T], mybir.dt.bfloat16, tag="qk")
                    vt = big.tile([128, NKC, HPC, DK + 1],
                                  mybir.dt.bfloat16, tag="vt")
                    nc.gpsimd.memset(qk[:], 0.02)
                    nc.gpsimd.memset(vt[:], 1.0)
                    tc._kx_preloaded['qkvt'] = (qk, vt)
                else:
                    tc._kx_preloaded['inputs'] = _emit_inputs(
                        nc, big, xt_d, wq_d, wo_d)
            emit = _emit_hsplit if hsplit else _emit
            if reps == 1:
                emit(nc, tc, pools, dram, opts)
            else:
                with tc.For_i(0, reps, 1):
                    emit(nc, tc, pools, dram, opts)

    nc.compile()
    return nc


def prep_inputs(x, w_qkv, w_o):
    """Host-side shard + layout prep. Returns in_maps for cores 0..7."""
    bf = ml_dtypes.bfloat16
    in_maps = []
    for c in range(NCORES):
        b, hh = c // 2, HPC * (c % 2)
        qrows = w_qkv[hh * DK:(hh + HPC) * DK]                    # [512, 1024]
        krows = w_qkv[D + hh * DK:D + (hh + HPC) * DK]
        vrows = w_qkv[2 * D + hh * DK:2 * D + (hh + HPC) * DK]
        wqt = np.concatenate([qrows, krows, vrows], 0).T          # [1024, 1536]
        in_maps.append({
            "xt": np.ascontiguousarray(x[b].T).astype(bf).reshape(8, 128, T),
            "wq": wqt.astype(bf).reshape(8, 128, 1536),
            "wo": np.ascontiguousarray(w_o[:, hh * DK:(hh + HPC) * DK].T)
                    .astype(bf).reshape(4, 128, D),
        })
    return in_maps


def get_nc(reps=1, opts=()):
    key = ("nc", reps, tuple(opts))
    if key not in _cache:
        _cache[key] = _build(reps, tuple(opts))
    return _cache[key]


DEFAULT_OPTS = ()


def kernel(x, w_qkv, w_o):
    from concourse.bass_utils import run_bass_kernel_spmd

    nc = get_nc(1, DEFAULT_OPTS)
    in_maps = prep_inputs(np.asarray(x, dtype=np.float32),
                          np.asarray(w_qkv, dtype=np.float32),
                          np.asarray(w_o, dtype=np.float32))
    try:
        res = run_bass_kernel_spmd(nc, in_maps, core_ids=list(range(NCORES)))
    except Exception:
        # transient device faults (e.g. NRT_EXEC_UNIT_UNRECOVERABLE) have
        # been observed once on an otherwise-correct build; retry once
        res = run_bass_kernel_spmd(nc, in_maps, core_ids=list(range(NCORES)))
    out = np.empty((B, T, D), np.float32)
    for b in range(B):
        out[b] = (res.results[2 * b]["out"].astype(np.float32)
                  + res.results[2 * b + 1]["out"].astype(np.float32))
    return out

